# revision 7
# baseline (speedup 1.0000x reference)
"""Trainium2 Bass kernel for nn_Net_18021682774696 (MTGNN-style GNN).

Strategy: data-parallel over batch B=8 -> 1 batch per NeuronCore.
Per core, the three graph-propagation branches (static, dy, dyT) run
concurrently as PE column strips (tile_position=(0,32j), M=32 each).
Adjacency matrices are host-normalized ((A+I) row-normalized, (1-alpha)
folded in, transposed to the rhs layout) and cast to bf16; dy-derived
ones are fully SBUF-resident, the shared static one is partially
resident / partially streamed.  All channel-mix convs, layernorm, and
the skip/end head run on-chip; the skip/end projections are
pre-collapsed on the host (endW @ skipW_i) so the S=256 dim vanishes.
"""
import sys
import os

sys.path.insert(0, '/opt/trn_rl_repo')

import numpy as np
import ml_dtypes

# ----------------------------------------------------------------------------
# Patches: this container's walrus accepts only ONE sem-wait per instruction.
# Split multi-wait instructions (tile attaches one wait per processor).
# ----------------------------------------------------------------------------
import concourse.bass as bass
import concourse.mybir as mybir
import concourse.tile as tile
from concourse.vector_clock import ScopedClock
from concourse.bass_utils import run_bass_kernel_spmd


def _drain_and_barrier_split(self, tick_clock, wait_clock):
    nc = self.nc
    drain_inst = nc.sync.drain()
    wait_clock.add_sem_waits(
        drain_inst.ins, ScopedClock({None: tick_clock.global_clock})
    )
    waits = list(drain_inst.ins.sync_info.on_wait)
    if len(waits) > 1:
        si = drain_inst.ins.sync_info
        si.on_wait = [waits[0]]
        drain_inst.ins.sync_info = si
        for w in waits[1:]:
            d2 = nc.sync.drain()
            d2.ins.sync_info = mybir.SyncInfo(on_wait=[w], on_update=[])

    nc.all_engine_barrier()
    assert self.sems is not None
    popped = nc._tile_sem_poison_stack.pop()
    assert popped is self._sem_poison
    nc.clear_and_free_semaphores(list(self.sems.allocated().values()))
    nc.all_engine_barrier()


tile.TileContext._drain_and_barrier = _drain_and_barrier_split

_orig_postorder = tile.postorder_instruction_blocks
_split_counter = [0]


def _split_multi_waits(ordered, start_bb_name, postordered_blocks):
    for bb_name, insts in ordered.items():
        new_list = []
        for inst in insts:
            si = getattr(inst, 'sync_info', None)
            waits = list(si.on_wait) if si is not None else []
            if len(waits) > 1:
                for w in waits[:-1]:
                    _split_counter[0] += 1
                    nop = mybir.InstNoOp(
                        name=f"I-waitsplit-{_split_counter[0]}", ins=[], outs=[])
                    nop.engine = inst.engine
                    nop.sync_info = mybir.SyncInfo(on_wait=[w], on_update=[])
                    new_list.append(nop)
                si.on_wait = [waits[-1]]
                inst.sync_info = si
            new_list.append(inst)
        ordered[bb_name] = new_list
    return _orig_postorder(ordered, start_bb_name, postordered_blocks)


tile.postorder_instruction_blocks = _split_multi_waits

# ----------------------------------------------------------------------------
# Model constants (hardcoded from the problem spec)
# ----------------------------------------------------------------------------
B, N, C, H, S, T = 8, 2048, 32, 128, 256, 12
LAYERS, NUM_TCN, GDEP = 3, 2, 2
ALPHA, EPS = 0.05, 1e-5
BF16 = mybir.dt.bfloat16
F32 = mybir.dt.float32
KCH = N // 128          # 16 contraction chunks
NCH = N // 512          # 4 psum n-chunks
R_STATIC = 7            # resident k-chunks of the static adjacency
NBF = ml_dtypes.bfloat16

_prog_cache = {}


def _build(has_affine: bool):
    nc = bass.Bass(trn_type="TRN2", name="gnn_mp")
    ts, AF, ALU = bass.ts, mybir.ActivationFunctionType, mybir.AluOpType

    # ---- DRAM I/O ----
    adj = [nc.dram_tensor(f"adj{j}", [N, N], BF16, kind="ExternalInput")
           for j in range(3)]                      # 0 static, 1 dy, 2 dyT
    x0_d = nc.dram_tensor("x0", [C, N], BF16, kind="ExternalInput")
    x0T_d = nc.dram_tensor("x0T", [N, C], BF16, kind="ExternalInput")
    embs_d = nc.dram_tensor("embs", [96, N], BF16, kind="ExternalInput")
    id3_d = nc.dram_tensor("id3", [96, 32], BF16, kind="ExternalInput")
    alphaI_d = nc.dram_tensor("alphaI", [32, 32], BF16, kind="ExternalInput")
    tcnW_d = nc.dram_tensor("tcnW", [H, LAYERS * 4 * H], BF16, kind="ExternalInput")
    tcnB_d = nc.dram_tensor("tcnB", [H, LAYERS * 4], F32, kind="ExternalInput")
    g1s_d = nc.dram_tensor("g1s", [96, LAYERS * 32], BF16, kind="ExternalInput")
    g2s_d = nc.dram_tensor("g2s", [96, LAYERS * 32], BF16, kind="ExternalInput")
    ga_d = nc.dram_tensor("ga", [32, LAYERS * 32], BF16, kind="ExternalInput")
    gb_d = nc.dram_tensor("gb", [32, LAYERS], F32, kind="ExternalInput")
    ew_d = nc.dram_tensor("ew", [H, LAYERS * T], BF16, kind="ExternalInput")
    ewe_d = nc.dram_tensor("ewe", [C, T], BF16, kind="ExternalInput")
    cb_d = nc.dram_tensor("cb", [T, 1], F32, kind="ExternalInput")
    if has_affine:
        nw_d = nc.dram_tensor("nw", [C, LAYERS * N], F32, kind="ExternalInput")
        nb_d = nc.dram_tensor("nb", [C, LAYERS * N], F32, kind="ExternalInput")
    out_d = nc.dram_tensor("out", [T, N], F32, kind="ExternalOutput")

    with tile.TileContext(nc) as tc:
        with (
            tc.tile_pool(name="adjp", bufs=1) as adjp,
            tc.tile_pool(name="cst", bufs=1) as cst,
            tc.tile_pool(name="wk", bufs=1) as wk,
            tc.tile_pool(name="st", bufs=2) as stp,
            tc.tile_pool(name="sc", bufs=2) as scp,
            tc.tile_pool(name="ps", bufs=8, space="PSUM") as ps,
        ):
            # ---- persistent SBUF ----
            a1 = adjp.tile([128, KCH, N], BF16, name="a1")
            a2 = adjp.tile([128, KCH, N], BF16, name="a2")
            a0 = adjp.tile([128, R_STATIC, N], BF16, name="a0")
            hid = cst.tile([H, N], BF16, name="hid")        # x rows 0:32, embs 32:128
            id3 = cst.tile([96, 32], BF16, name="id3")
            alphaI = cst.tile([32, 32], BF16, name="alphaI")
            tcnW = cst.tile([H, LAYERS * 4 * H], BF16, name="tcnW")
            tcnB = cst.tile([H, LAYERS * 4], F32, name="tcnB")
            g1s = cst.tile([96, LAYERS * 32], BF16, name="g1s")
            g2s = cst.tile([96, LAYERS * 32], BF16, name="g2s")
            ga = cst.tile([32, LAYERS * 32], BF16, name="ga")
            gb = cst.tile([32, LAYERS], F32, name="gb")
            ew = cst.tile([H, LAYERS * T], BF16, name="ew")
            ewe = cst.tile([C, T], BF16, name="ewe")
            cb = cst.tile([T, 1], F32, name="cb")
            ones1 = cst.tile([1, 32], F32, name="ones1")
            if has_affine:
                nw = cst.tile([C, LAYERS * N], F32, name="nw")
                nb = cst.tile([C, LAYERS * N], F32, name="nb")

            x_wc = wk.tile([128, KCH, 32], BF16, name="x_wc")
            h1_wc = [wk.tile([128, KCH, 32], BF16, name=f"h1wc{j}") for j in range(3)]
            stack1 = wk.tile([96, N], BF16, name="stack1")
            stack2 = wk.tile([96, N], BF16, name="stack2")
            htcn1 = wk.tile([H, N], BF16, name="htcn1")
            out_acc = wk.tile([T, N], F32, name="out_acc")
            sums = wk.tile([32, 2 * NCH], F32, name="sums")
            stat = wk.tile([1, 4], F32, name="stat")
            bc = wk.tile([32, 2], F32, name="bc")
            eps_t = wk.tile([1, 1], F32, name="eps_t")

            # ---- load constants then adjacency ----
            nc.sync.dma_start(hid[0:C, :], x0_d[:])
            nc.sync.dma_start(hid[C:H, :], embs_d[:])
            nc.sync.dma_start(id3[:], id3_d[:])
            nc.sync.dma_start(alphaI[:], alphaI_d[:])
            nc.sync.dma_start(tcnW[:], tcnW_d[:])
            nc.sync.dma_start(tcnB[:], tcnB_d[:])
            nc.sync.dma_start(g1s[:], g1s_d[:])
            nc.sync.dma_start(g2s[:], g2s_d[:])
            nc.sync.dma_start(ga[:], ga_d[:])
            nc.sync.dma_start(gb[:], gb_d[:])
            nc.sync.dma_start(ew[:], ew_d[:])
            nc.sync.dma_start(ewe[:], ewe_d[:])
            nc.sync.dma_start(cb[:], cb_d[:])
            nc.sync.dma_start(x_wc[:], x0T_d.rearrange("(k p) c -> p k c", p=128))
            if has_affine:
                nc.sync.dma_start(nw[:], nw_d[:])
                nc.sync.dma_start(nb[:], nb_d[:])
            nc.vector.memset(ones1[:], 1.0)
            nc.vector.memset(eps_t[:], EPS)
            nc.vector.memset(out_acc[:], 0.0)
            for k in range(KCH):
                nc.sync.dma_start(a1[:, k, :], adj[1][ts(k, 128), :])
            for k in range(KCH):
                nc.sync.dma_start(a2[:, k, :], adj[2][ts(k, 128), :])
            for k in range(R_STATIC):
                nc.sync.dma_start(a0[:, k, :], adj[0][ts(k, 128), :])

            def stream_static():
                """DMA the non-resident static chunks for one use."""
                tiles = []
                for k in range(R_STATIC, KCH):
                    t = stp.tile([128, N], BF16, tag="s0s", name="s0s")
                    nc.scalar.dma_start(t[:], adj[0][ts(k, 128), :])
                    tiles.append((k, t))
                return tiles

            def prop_step(stationaries, strips, streamed):
                """One propagation hop: psum[j] = alpha*x + sum_k A_j^k h^k.
                stationaries[j]: tile [128, KCH, 32]; strips: list of j.
                Returns psum tiles [128, 512] x NCH (strip j at part 32j)."""
                pt = [ps.tile([128, 512], F32, tag="ps", name=f"pt{_n}") for _n in range(NCH)]
                for n in range(NCH):
                    for j in strips:
                        nc.tensor.matmul(
                            pt[n][32 * j:32 * j + 32, :], alphaI[:],
                            hid[0:32, ts(n, 512)],
                            start=True, stop=False, tile_position=(0, 32 * j))
                st_map = dict(streamed) if streamed else {}
                for k in range(KCH):
                    for j in strips:
                        if j == 0:
                            rk = a0[:, k, :] if k < R_STATIC else st_map[k][:]
                        else:
                            rk = (a1 if j == 1 else a2)[:, k, :]
                        for n in range(NCH):
                            nc.tensor.matmul(
                                pt[n][32 * j:32 * j + 32, :],
                                stationaries[j][:, k, :],
                                rk[:, ts(n, 512)],
                                start=False, stop=(k == KCH - 1),
                                tile_position=(0, 32 * j))
                return pt

            def hop_copy(pt, dst):
                for j in range(3):
                    for n in range(NCH):
                        nc.vector.tensor_copy(
                            dst[32 * j:32 * j + 32, ts(n, 512)],
                            pt[n][32 * j:32 * j + 32, :])

            def transpose_to(src_ap_fn, dst, tpos_row):
                """dst [128,KCH,32] <- transpose of a [32, N] tensor."""
                for q in range(KCH // 4):
                    tp = ps.tile([128, 128], F32, tag="ps", name="tp")
                    for r in range(4):
                        k = 4 * q + r
                        nc.tensor.matmul(
                            tp[:, 32 * r:32 * r + 32],
                            src_ap_fn(k), id3[tpos_row:tpos_row + 32, :],
                            start=True, stop=True, tile_position=(tpos_row, 0))
                    nc.scalar.copy(
                        dst[:, 4 * q:4 * q + 4, :].rearrange("p a b -> p (a b)"),
                        tp[:])

            # ================= layers =================
            for i in range(LAYERS):
                # ---- TCN units ----
                src = hid
                for u in range(NUM_TCN):
                    wf = tcnW[:, ts(i * 4 + 2 * u, H)]
                    wg = tcnW[:, ts(i * 4 + 2 * u + 1, H)]
                    bf = tcnB[:, i * 4 + 2 * u:i * 4 + 2 * u + 1]
                    bg = tcnB[:, i * 4 + 2 * u + 1:i * 4 + 2 * u + 2]
                    dst_full = htcn1 if u == 0 else None
                    for n in range(NCH):
                        pf = ps.tile([H, 512], F32, tag="ps", name="pf")
                        pg = ps.tile([H, 512], F32, tag="ps", name="pg")
                        nc.tensor.matmul(pf[:], wf, src[:, ts(n, 512)],
                                         start=True, stop=True)
                        nc.tensor.matmul(pg[:], wg, src[:, ts(n, 512)],
                                         start=True, stop=True)
                        tf = scp.tile([H, 512], BF16, tag="tf", name="tf")
                        tg = scp.tile([H, 512], BF16, tag="tg", name="tg")
                        nc.scalar.activation(tf[:], pf[:], AF.Tanh, bias=bf)
                        nc.scalar.activation(tg[:], pg[:], AF.Sigmoid, bias=bg)
                        if u == 0:
                            nc.vector.tensor_mul(dst_full[:, ts(n, 512)], tf[:], tg[:])
                        else:
                            h2c = scp.tile([H, 512], BF16, tag="h2c", name="h2c")
                            nc.vector.tensor_mul(h2c[:], tf[:], tg[:])
                            # skip conv on this chunk, accumulate to out_acc
                            pk = ps.tile([T, 512], F32, tag="ps", name="pk")
                            nc.tensor.matmul(pk[:], ew[:, ts(i, T)], h2c[:],
                                             start=True, stop=True)
                            nc.vector.tensor_add(out_acc[:, ts(n, 512)],
                                                 out_acc[:, ts(n, 512)], pk[:])
                    src = htcn1

                # ---- propagation step 1 ----
                strm1 = stream_static()
                if i == 0:
                    pt1 = prop_step([None, x_wc, x_wc], [1, 2], None)
                    # static strip afterwards (its adjacency loads last)
                    pt1b = pt1
                    for n in range(NCH):
                        nc.tensor.matmul(
                            pt1b[n][0:32, :], alphaI[:], hid[0:32, ts(n, 512)],
                            start=True, stop=False, tile_position=(0, 0))
                    sm1 = dict(strm1)
                    for k in range(KCH):
                        rhs = a0[:, k, :] if k < R_STATIC else sm1[k][:]
                        for n in range(NCH):
                            nc.tensor.matmul(
                                pt1b[n][0:32, :], x_wc[:, k, :],
                                rhs[:, ts(n, 512)],
                                start=False, stop=(k == KCH - 1),
                                tile_position=(0, 0))
                else:
                    pt1 = prop_step([x_wc, x_wc, x_wc], [0, 1, 2], strm1)
                hop_copy(pt1, stack1)

                # ---- transposes of h1 (per branch) ----
                for j in range(3):
                    transpose_to(lambda k, j=j: stack1[32 * j:32 * j + 32, ts(k, 128)],
                                 h1_wc[j], 32 * j)

                # ---- propagation step 2 ----
                strm2 = stream_static()
                pt2 = prop_step(h1_wc, [0, 1, 2], strm2)
                hop_copy(pt2, stack2)

                # ---- mixprop mlps + residual + bias -> psum ----
                pm = [ps.tile([32, 512], F32, tag="ps", name=f"pm{_n}") for _n in range(NCH)]
                for n in range(NCH):
                    nc.tensor.matmul(pm[n][:], ga[:, ts(i, 32)],
                                     hid[0:32, ts(n, 512)], start=True, stop=False)
                    nc.tensor.matmul(pm[n][:], g1s[:, ts(i, 32)],
                                     stack1[:, ts(n, 512)], start=False, stop=False)
                    nc.tensor.matmul(pm[n][:], g2s[:, ts(i, 32)],
                                     stack2[:, ts(n, 512)], start=False, stop=True)
                    # += gb + residual (in place on psum)
                    nc.vector.scalar_tensor_tensor(
                        out=pm[n][:], in0=pm[n][:],
                        scalar=gb[:, i:i + 1], in1=hid[0:32, ts(n, 512)],
                        op0=ALU.add, op1=ALU.add)

                # ---- layernorm stats ----
                for n in range(NCH):
                    nc.vector.tensor_reduce(sums[:, n:n + 1], pm[n][:],
                                            mybir.AxisListType.X, ALU.add)
                    sq = scp.tile([32, 512], BF16, tag="sq", name="sq")
                    nc.scalar.activation(sq[:], pm[n][:], AF.Square,
                                         accum_out=sums[:, NCH + n:NCH + n + 1])
                tot = wk.tile([1, 2 * NCH], F32, name=f"tot{i}")
                nc.gpsimd.tensor_reduce(tot[:], sums[:], mybir.AxisListType.C, ALU.add)
                nc.vector.tensor_reduce(stat[:, 0:1], tot[:, 0:NCH],
                                        mybir.AxisListType.X, ALU.add)
                nc.vector.tensor_reduce(stat[:, 1:2], tot[:, NCH:2 * NCH],
                                        mybir.AxisListType.X, ALU.add)
                # mu = s1/(C*N); e2 = s2/(C*N); var = e2 - mu^2
                nc.vector.tensor_scalar_mul(stat[:, 0:1], stat[:, 0:1], 1.0 / (C * N))
                nc.vector.tensor_scalar_mul(stat[:, 1:2], stat[:, 1:2], 1.0 / (C * N))
                nc.vector.tensor_mul(stat[:, 2:3], stat[:, 0:1], stat[:, 0:1])
                nc.vector.tensor_sub(stat[:, 1:2], stat[:, 1:2], stat[:, 2:3])
                nc.scalar.activation(stat[:, 1:2], stat[:, 1:2], AF.Sqrt, bias=eps_t[:])
                nc.vector.reciprocal(stat[:, 1:2], stat[:, 1:2])
                pb = ps.tile([32, 2], F32, tag="ps", name="pb")
                nc.tensor.matmul(pb[:], ones1[:], stat[:, 0:2], start=True, stop=True)
                nc.vector.tensor_copy(bc[:], pb[:])

                # ---- normalize + (affine) + relu -> hid[0:32] ----
                for n in range(NCH):
                    if has_affine:
                        tmp = scp.tile([32, 512], F32, tag="tmp", name="tmp")
                        nc.vector.tensor_scalar(
                            out=tmp[:], in0=pm[n][:],
                            scalar1=bc[:, 0:1], scalar2=bc[:, 1:2],
                            op0=ALU.subtract, op1=ALU.mult)
                        nc.vector.tensor_mul(tmp[:], tmp[:],
                                             nw[:, bass.ds(i * N + n * 512, 512)])
                        nc.vector.tensor_add(tmp[:], tmp[:],
                                             nb[:, bass.ds(i * N + n * 512, 512)])
                        nc.vector.tensor_scalar_max(hid[0:32, ts(n, 512)], tmp[:], 0.0)
                    else:
                        tmp = scp.tile([32, 512], BF16, tag="tmp", name="tmp")
                        nc.vector.tensor_scalar(
                            out=tmp[:], in0=pm[n][:],
                            scalar1=bc[:, 0:1], scalar2=bc[:, 1:2],
                            op0=ALU.subtract, op1=ALU.mult)
                        nc.vector.tensor_scalar_max(hid[0:32, ts(n, 512)], tmp[:], 0.0)

                # ---- transpose new x for next layer ----
                if i < LAYERS - 1:
                    transpose_to(lambda k: hid[0:32, ts(k, 128)], x_wc, 0)

            # ---- final head: out = out_acc + EWE @ x3 + cb ----
            for n in range(NCH):
                pk = ps.tile([T, 512], F32, tag="ps", name="pk")
                nc.tensor.matmul(pk[:], ewe[:], hid[0:32, ts(n, 512)],
                                 start=True, stop=True)
                nc.vector.scalar_tensor_tensor(
                    out=out_acc[:, ts(n, 512)], in0=pk[:], scalar=cb[:],
                    in1=out_acc[:, ts(n, 512)], op0=ALU.add, op1=ALU.add)
            nc.sync.dma_start(out_d[:], out_acc[:])

    return nc


def _prep(inputs):
    """Host-side preprocessing -> per-core input maps."""
    f32 = np.float32
    x = inputs['x'].astype(f32).reshape(B, C, N)
    dy = inputs['dy_graph'].astype(f32)
    S_ = inputs['static_graph'].astype(f32)
    sp = inputs['spatial_emb'].astype(f32).reshape(B, 32, N)
    td = inputs['temporal_d_emb'].astype(f32).reshape(B, 32, N)
    tw = inputs['temporal_w_emb'].astype(f32).reshape(B, 32, N)

    sc = np.float32(1.0 - ALPHA)
    # static: rhs0[w,v] = (S^T + I)[w,v] * (1-a)/r0[v],  r0 = S.sum(1)+1
    r0 = S_.sum(1) + 1.0
    adj0 = ((S_.T + np.eye(N, dtype=f32)) * (sc / r0)[None, :]).astype(NBF)
    adj1 = np.empty((B, N, N), NBF)
    adj2 = np.empty((B, N, N), NBF)
    for b in range(B):
        d = dy[b]
        r1 = d.sum(1) + 1.0
        r2 = d.sum(0) + 1.0
        dT = np.ascontiguousarray(d.T)
        adj1[b] = ((dT + np.eye(N, dtype=f32)) * (sc / r1)[None, :]).astype(NBF)
        adj2[b] = ((d + np.eye(N, dtype=f32)) * (sc / r2)[None, :]).astype(NBF)

    id3 = np.zeros((96, 32), f32)
    for j in range(3):
        id3[32 * j:32 * j + 32] = np.eye(32)
    alphaI = (ALPHA * np.eye(32, dtype=f32)).astype(NBF)
    id3 = id3.astype(NBF)

    # TCN weights: lhsT = W^T laid out [cin, (layer,unit,fg)*cout]
    tcnW = np.zeros((H, LAYERS * 4 * H), f32)
    tcnB = np.zeros((H, LAYERS * 4), f32)
    for i in range(LAYERS):
        for u in range(NUM_TCN):
            tcnW[:, (i * 4 + 2 * u) * H:(i * 4 + 2 * u + 1) * H] = \
                inputs['enc_Wf'][i, u].astype(f32).T
            tcnW[:, (i * 4 + 2 * u + 1) * H:(i * 4 + 2 * u + 2) * H] = \
                inputs['enc_Wg'][i, u].astype(f32).T
            tcnB[:, i * 4 + 2 * u] = inputs['enc_bf'][i, u].astype(f32)
            tcnB[:, i * 4 + 2 * u + 1] = inputs['enc_bg'][i, u].astype(f32)

    gW = [inputs['g0_W'].astype(f32), inputs['g1_W'].astype(f32),
          inputs['g2_W'].astype(f32)]
    gB = [inputs['g0_b'].astype(f32), inputs['g1_b'].astype(f32),
          inputs['g2_b'].astype(f32)]
    g1s = np.zeros((96, LAYERS * 32), f32)
    g2s = np.zeros((96, LAYERS * 32), f32)
    ga = np.zeros((32, LAYERS * 32), f32)
    gb = np.zeros((32, LAYERS), f32)
    for i in range(LAYERS):
        for k in range(3):   # branch k: 0 static, 1 dy, 2 dyT
            g1s[32 * k:32 * k + 32, 32 * i:32 * i + 32] = gW[k][i][:, 32:64].T
            g2s[32 * k:32 * k + 32, 32 * i:32 * i + 32] = gW[k][i][:, 64:96].T
            ga[:, 32 * i:32 * i + 32] += gW[k][i][:, 0:32].T
            gb[:, i] += gB[k][i]

    endW = inputs['end_W'].astype(f32)
    ew = np.zeros((H, LAYERS * T), f32)
    for i in range(LAYERS):
        ew[:, i * T:(i + 1) * T] = (endW @ inputs['skip_W'][i].astype(f32)).T
    ewe = (endW @ inputs['skipE_W'].astype(f32)).T
    cb = (endW @ (inputs['skip_b'].astype(f32).sum(0)
                  + inputs['skipE_b'].astype(f32))
          + inputs['end_b'].astype(f32)).reshape(T, 1)

    nw = inputs['norm_w'].astype(f32).reshape(LAYERS, C, N)
    nbb = inputs['norm_b'].astype(f32).reshape(LAYERS, C, N)
    has_affine = not (np.all(nw == 1.0) and np.all(nbb == 0.0))

    shared = {
        "adj0": adj0, "id3": id3, "alphaI": alphaI,
        "tcnW": tcnW.astype(NBF), "tcnB": tcnB,
        "g1s": g1s.astype(NBF), "g2s": g2s.astype(NBF),
        "ga": ga.astype(NBF), "gb": gb,
        "ew": ew.astype(NBF), "ewe": ewe.astype(NBF), "cb": cb,
        "embs": None,  # per-core below
    }
    if has_affine:
        shared["nw"] = np.concatenate([nw[i] for i in range(LAYERS)], 1)
        shared["nb"] = np.concatenate([nbb[i] for i in range(LAYERS)], 1)

    in_maps = []
    for b in range(B):
        m = dict(shared)
        m["embs"] = np.concatenate([sp[b], td[b], tw[b]], 0).astype(NBF)
        m["x0"] = x[b].astype(NBF)
        m["x0T"] = np.ascontiguousarray(x[b].T).astype(NBF)
        m["adj1"] = adj1[b]
        m["adj2"] = adj2[b]
        in_maps.append(m)
    return in_maps, has_affine


LAST_EXEC_NS = None


def _install_profile_hook():
    import types
    import antenv
    if 'antenv.axon_hooks' not in sys.modules:
        mod = types.ModuleType('antenv.axon_hooks')
        holder = {}
        mod.set_axon_ntff_profile_hook = lambda h: holder.__setitem__('h', h)
        mod.get_axon_ntff_profile_hook = lambda: holder.get('h')
        sys.modules['antenv.axon_hooks'] = mod
        antenv.axon_hooks = mod
        from trn_agent_boot.trn_boot import _ntff_profile_via_ctypes
        mod.set_axon_ntff_profile_hook(
            _ntff_profile_via_ctypes('/opt/axon/libaxon_pjrt.so'))
    import concourse.bass_utils as bu
    bu.upload_artifacts = lambda tmpdir: tmpdir


def kernel(**inputs):
    global LAST_EXEC_NS
    in_maps, has_affine = _prep(inputs)
    if has_affine not in _prog_cache:
        _prog_cache[has_affine] = _build(has_affine)
    nc = _prog_cache[has_affine]
    trace = bool(os.environ.get("KERNEL_TRACE"))
    if trace:
        _install_profile_hook()
    res = run_bass_kernel_spmd(nc, in_maps, core_ids=list(range(B)), trace=trace)
    LAST_EXEC_NS = res.exec_time_ns
    out = np.stack([res.results[b]["out"] for b in range(B)], 0)
    return out.reshape(B, T, N, 1).astype(np.float32)


# revision 9
# speedup vs baseline: 1.1608x; 1.1608x over previous
"""Trainium2 Bass kernel for nn_Net_18021682774696 (MTGNN-style GNN).

Strategy: data-parallel over batch B=8 -> 1 batch per NeuronCore.
Per core, the three graph-propagation branches (static, dy, dyT) run
concurrently as PE column strips (tile_position=(0,32j), M=32 each).
Adjacency matrices are host-normalized ((A+I) row-normalized, (1-alpha)
folded in, transposed to the rhs layout) and cast to bf16; dy-derived
ones are fully SBUF-resident, the shared static one is partially
resident / partially streamed.  All channel-mix convs, layernorm, and
the skip/end head run on-chip; the skip/end projections are
pre-collapsed on the host (endW @ skipW_i) so the S=256 dim vanishes.
"""
import sys
import os

sys.path.insert(0, '/opt/trn_rl_repo')

import numpy as np
import ml_dtypes

# ----------------------------------------------------------------------------
# Patches: this container's walrus accepts only ONE sem-wait per instruction.
# Split multi-wait instructions (tile attaches one wait per processor).
# ----------------------------------------------------------------------------
import concourse.bass as bass
import concourse.mybir as mybir
import concourse.tile as tile
from concourse.vector_clock import ScopedClock
from concourse.bass_utils import run_bass_kernel_spmd


def _drain_and_barrier_split(self, tick_clock, wait_clock):
    nc = self.nc
    drain_inst = nc.sync.drain()
    wait_clock.add_sem_waits(
        drain_inst.ins, ScopedClock({None: tick_clock.global_clock})
    )
    waits = list(drain_inst.ins.sync_info.on_wait)
    if len(waits) > 1:
        si = drain_inst.ins.sync_info
        si.on_wait = [waits[0]]
        drain_inst.ins.sync_info = si
        for w in waits[1:]:
            d2 = nc.sync.drain()
            d2.ins.sync_info = mybir.SyncInfo(on_wait=[w], on_update=[])

    nc.all_engine_barrier()
    assert self.sems is not None
    popped = nc._tile_sem_poison_stack.pop()
    assert popped is self._sem_poison
    nc.clear_and_free_semaphores(list(self.sems.allocated().values()))
    nc.all_engine_barrier()


tile.TileContext._drain_and_barrier = _drain_and_barrier_split

_orig_postorder = tile.postorder_instruction_blocks
_split_counter = [0]


def _split_multi_waits(ordered, start_bb_name, postordered_blocks):
    for bb_name, insts in ordered.items():
        new_list = []
        for inst in insts:
            si = getattr(inst, 'sync_info', None)
            waits = list(si.on_wait) if si is not None else []
            if len(waits) > 1:
                for w in waits[:-1]:
                    _split_counter[0] += 1
                    nop = mybir.InstNoOp(
                        name=f"I-waitsplit-{_split_counter[0]}", ins=[], outs=[])
                    nop.engine = inst.engine
                    nop.sync_info = mybir.SyncInfo(on_wait=[w], on_update=[])
                    new_list.append(nop)
                si.on_wait = [waits[-1]]
                inst.sync_info = si
            new_list.append(inst)
        ordered[bb_name] = new_list
    return _orig_postorder(ordered, start_bb_name, postordered_blocks)


tile.postorder_instruction_blocks = _split_multi_waits

# ----------------------------------------------------------------------------
# Model constants (hardcoded from the problem spec)
# ----------------------------------------------------------------------------
B, N, C, H, S, T = 8, 2048, 32, 128, 256, 12
LAYERS, NUM_TCN, GDEP = 3, 2, 2
ALPHA, EPS = 0.05, 1e-5
BF16 = mybir.dt.bfloat16
F32 = mybir.dt.float32
KCH = N // 128          # 16 contraction chunks
NCH = N // 512          # 4 psum n-chunks
R_STATIC = 8            # resident k-chunks of the static adjacency
NBF = ml_dtypes.bfloat16

_prog_cache = {}


def _build(has_affine: bool):
    global R_STATIC
    R_STATIC = 4 if has_affine else 8
    nc = bass.Bass(trn_type="TRN2", name="gnn_mp")
    ts, AF, ALU = bass.ts, mybir.ActivationFunctionType, mybir.AluOpType

    # ---- DRAM I/O ----
    adj = [nc.dram_tensor(f"adj{j}", [N, N], BF16, kind="ExternalInput")
           for j in range(3)]                      # 0 static, 1 dy, 2 dyT
    x0_d = nc.dram_tensor("x0", [C, N], BF16, kind="ExternalInput")
    x0T_d = nc.dram_tensor("x0T", [N, C], BF16, kind="ExternalInput")
    embs_d = nc.dram_tensor("embs", [96, N], BF16, kind="ExternalInput")
    id3_d = nc.dram_tensor("id3", [96, 32], BF16, kind="ExternalInput")
    alphaI_d = nc.dram_tensor("alphaI", [32, 32], BF16, kind="ExternalInput")
    tcnW_d = nc.dram_tensor("tcnW", [H, LAYERS * 4 * H], BF16, kind="ExternalInput")
    tcnB_d = nc.dram_tensor("tcnB", [H, LAYERS * 4], F32, kind="ExternalInput")
    g1s_d = nc.dram_tensor("g1s", [96, LAYERS * 32], BF16, kind="ExternalInput")
    g2s_d = nc.dram_tensor("g2s", [96, LAYERS * 32], BF16, kind="ExternalInput")
    ga_d = nc.dram_tensor("ga", [32, LAYERS * 32], BF16, kind="ExternalInput")
    gb_d = nc.dram_tensor("gb", [32, LAYERS], F32, kind="ExternalInput")
    ew_d = nc.dram_tensor("ew", [H, LAYERS * T], BF16, kind="ExternalInput")
    ewe_d = nc.dram_tensor("ewe", [C, T], BF16, kind="ExternalInput")
    cb_d = nc.dram_tensor("cb", [T, 1], F32, kind="ExternalInput")
    if has_affine:
        nw_d = nc.dram_tensor("nw", [C, LAYERS * N], BF16, kind="ExternalInput")
        nb_d = nc.dram_tensor("nb", [C, LAYERS * N], BF16, kind="ExternalInput")
    out_d = nc.dram_tensor("out", [T, N], F32, kind="ExternalOutput")

    with tile.TileContext(nc) as tc:
        with (
            tc.tile_pool(name="adjp", bufs=1) as adjp,
            tc.tile_pool(name="cst", bufs=1) as cst,
            tc.tile_pool(name="wk", bufs=1) as wk,
            tc.tile_pool(name="st", bufs=3) as stp,
            tc.tile_pool(name="sc", bufs=2) as scp,
            tc.tile_pool(name="hc", bufs=1) as hcp,
            tc.tile_pool(name="ps", bufs=8, space="PSUM") as ps,
        ):
            # ---- persistent SBUF ----
            a1 = adjp.tile([128, KCH, N], BF16, name="a1")
            a2 = adjp.tile([128, KCH, N], BF16, name="a2")
            a0 = adjp.tile([128, R_STATIC, N], BF16, name="a0")
            hid = cst.tile([H, N], BF16, name="hid")        # x rows 0:32, embs 32:128
            id3 = cst.tile([96, 32], BF16, name="id3")
            alphaI = cst.tile([32, 32], BF16, name="alphaI")
            tcnW = cst.tile([H, LAYERS * 4 * H], BF16, name="tcnW")
            tcnB = cst.tile([H, LAYERS * 4], F32, name="tcnB")
            g1s = cst.tile([96, LAYERS * 32], BF16, name="g1s")
            g2s = cst.tile([96, LAYERS * 32], BF16, name="g2s")
            ga = cst.tile([32, LAYERS * 32], BF16, name="ga")
            gb = cst.tile([32, LAYERS], F32, name="gb")
            ew = cst.tile([H, LAYERS * T], BF16, name="ew")
            ewe = cst.tile([C, T], BF16, name="ewe")
            cb = cst.tile([T, 1], F32, name="cb")
            ones1 = cst.tile([1, 32], F32, name="ones1")
            if has_affine:
                nw = cst.tile([C, LAYERS * N], BF16, name="nw")
                nb = cst.tile([C, LAYERS * N], BF16, name="nb")

            x_wc = wk.tile([128, KCH, 32], BF16, name="x_wc")
            h1_wc = [wk.tile([128, KCH, 32], BF16, name=f"h1wc{j}") for j in range(3)]
            stack1 = wk.tile([96, N], BF16, name="stack1")
            stack2 = wk.tile([96, N], BF16, name="stack2")
            out_acc = wk.tile([T, N], F32, name="out_acc")
            sums = wk.tile([32, 2 * NCH], F32, name="sums")
            stat = wk.tile([1, 4], F32, name="stat")
            bc = wk.tile([32, 2], F32, name="bc")
            eps_t = wk.tile([1, 1], F32, name="eps_t")

            # ---- load constants then adjacency ----
            nc.sync.dma_start(hid[0:C, :], x0_d[:])
            nc.sync.dma_start(hid[C:H, :], embs_d[:])
            nc.sync.dma_start(id3[:], id3_d[:])
            nc.sync.dma_start(alphaI[:], alphaI_d[:])
            nc.sync.dma_start(tcnW[:], tcnW_d[:])
            nc.sync.dma_start(tcnB[:], tcnB_d[:])
            nc.sync.dma_start(g1s[:], g1s_d[:])
            nc.sync.dma_start(g2s[:], g2s_d[:])
            nc.sync.dma_start(ga[:], ga_d[:])
            nc.sync.dma_start(gb[:], gb_d[:])
            nc.sync.dma_start(ew[:], ew_d[:])
            nc.sync.dma_start(ewe[:], ewe_d[:])
            nc.sync.dma_start(cb[:], cb_d[:])
            nc.sync.dma_start(x_wc[:], x0T_d.rearrange("(k p) c -> p k c", p=128))
            if has_affine:
                nc.sync.dma_start(nw[:], nw_d[:])
                nc.sync.dma_start(nb[:], nb_d[:])
            nc.vector.memset(ones1[:], 1.0)
            nc.vector.memset(eps_t[:], EPS)
            nc.vector.memset(out_acc[:], 0.0)
            for k in range(KCH):
                nc.sync.dma_start(a1[:, k, :], adj[1][ts(k, 128), :])
            for k in range(KCH):
                nc.sync.dma_start(a2[:, k, :], adj[2][ts(k, 128), :])
            for k in range(R_STATIC):
                nc.sync.dma_start(a0[:, k, :], adj[0][ts(k, 128), :])

            def stream_static(which):
                """DMA the non-resident static chunks (half-tiles) for one use."""
                tiles = {}
                for k in range(R_STATIC, KCH):
                    for h in range(2):
                        t = stp.tile([128, N // 2], BF16, tag="s0s", name="s0s")
                        nc.scalar.dma_start(
                            t[:], adj[0][ts(k, 128), bass.ds(h * (N // 2), N // 2)])
                        tiles[(k, h)] = t
                return tiles

            def prop_step(stationaries, strips, st_map, korder=None):
                """One propagation hop: psum[j] = alpha*x + sum_k A_j^k h^k."""
                pt = [ps.tile([128, 512], F32, tag="ps", name=f"pt{_n}")
                      for _n in range(NCH)]
                for n in range(NCH):
                    for j in strips:
                        nc.tensor.matmul(
                            pt[n][32 * j:32 * j + 32, :], alphaI[:],
                            hid[0:32, ts(n, 512)],
                            start=True, stop=False, tile_position=(0, 32 * j))
                korder = korder or list(range(KCH))
                for ki, k in enumerate(korder):
                    last = ki == KCH - 1
                    for j in strips:
                        for n in range(NCH):
                            if j == 0 and k >= R_STATIC:
                                rk = st_map[(k, n // 2)][:, ts(n % 2, 512)]
                            elif j == 0:
                                rk = a0[:, k, ts(n, 512)]
                            else:
                                rk = (a1 if j == 1 else a2)[:, k, ts(n, 512)]
                            nc.tensor.matmul(
                                pt[n][32 * j:32 * j + 32, :],
                                stationaries[j][:, k, :], rk,
                                start=False, stop=last,
                                tile_position=(0, 32 * j))
                return pt

            def transpose_grp(src_ap_fn, dst, tpos_row, q):
                """dst[:, 4q:4q+4, :] <- transpose of src cols 512q..512q+512."""
                tp = ps.tile([128, 128], F32, tag="ps", name="tp")
                for r in range(4):
                    k = 4 * q + r
                    nc.tensor.matmul(
                        tp[:, 32 * r:32 * r + 32],
                        src_ap_fn(k), id3[tpos_row:tpos_row + 32, :],
                        start=True, stop=True, tile_position=(tpos_row, 0))
                nc.vector.tensor_copy(
                    dst[:, 4 * q:4 * q + 4, :].rearrange("p a b -> p (a b)"),
                    tp[:])

            # ================= layers =================
            for i in range(LAYERS):
                # stream the static chunks for both hops up front; the DMA
                # queue drains them in FIFO order as slots free up.
                strm1 = stream_static(0)
                strm2 = stream_static(1)

                # ---- TCN units (chunk-pipelined) ----
                htc = [None] * NCH
                for u in range(NUM_TCN):
                    wf = tcnW[:, ts(i * 4 + 2 * u, H)]
                    wg = tcnW[:, ts(i * 4 + 2 * u + 1, H)]
                    bf = tcnB[:, i * 4 + 2 * u:i * 4 + 2 * u + 1]
                    bg = tcnB[:, i * 4 + 2 * u + 1:i * 4 + 2 * u + 2]
                    for n in range(NCH):
                        src = hid[:, ts(n, 512)] if u == 0 else htc[n][:]
                        pf = ps.tile([H, 512], F32, tag="ps", name="pf")
                        pg = ps.tile([H, 512], F32, tag="ps", name="pg")
                        nc.tensor.matmul(pf[:], wf, src, start=True, stop=True)
                        nc.tensor.matmul(pg[:], wg, src, start=True, stop=True)
                        tf = scp.tile([H, 512], BF16, tag="tf", name="tf")
                        tg = scp.tile([H, 512], BF16, tag="tg", name="tg")
                        nc.scalar.activation(tf[:], pf[:], AF.Tanh, bias=bf)
                        nc.scalar.activation(tg[:], pg[:], AF.Sigmoid, bias=bg)
                        if u == 0:
                            htc[n] = hcp.tile([H, 512], BF16, tag=f"htc{n}",
                                              name="htc")
                            nc.vector.tensor_mul(htc[n][:], tf[:], tg[:])
                        else:
                            h2c = scp.tile([H, 512], BF16, tag="h2c", name="h2c")
                            nc.vector.tensor_mul(h2c[:], tf[:], tg[:])
                            pk = ps.tile([T, 512], F32, tag="ps", name="pk")
                            nc.tensor.matmul(pk[:], ew[:, ts(i, T)], h2c[:],
                                             start=True, stop=True)
                            nc.vector.tensor_add(out_acc[:, ts(n, 512)],
                                                 out_acc[:, ts(n, 512)], pk[:])

                # ---- propagation step 1 ----
                if i == 0:
                    # dy-derived strips first (their chunks DMA first), the
                    # static strip after, streamed chunks before resident
                    # ones (resident a0 is last in the load queue).
                    pt1 = prop_step([None, x_wc, x_wc], [1, 2], None)
                    for n in range(NCH):
                        nc.tensor.matmul(
                            pt1[n][0:32, :], alphaI[:], hid[0:32, ts(n, 512)],
                            start=True, stop=False, tile_position=(0, 0))
                    korder = list(range(R_STATIC, KCH)) + list(range(R_STATIC))
                    for ki, k in enumerate(korder):
                        for n in range(NCH):
                            rk = (strm1[(k, n // 2)][:, ts(n % 2, 512)]
                                  if k >= R_STATIC else a0[:, k, ts(n, 512)])
                            nc.tensor.matmul(
                                pt1[n][0:32, :], x_wc[:, k, :], rk,
                                start=False, stop=(ki == KCH - 1),
                                tile_position=(0, 0))
                else:
                    pt1 = prop_step([x_wc, x_wc, x_wc], [0, 1, 2], strm1)

                # hop copy + h1 transposes, interleaved per chunk
                for n in range(NCH):
                    nc.vector.tensor_copy(stack1[:, ts(n, 512)], pt1[n][0:96, :])
                    for j in range(3):
                        transpose_grp(
                            lambda k, j=j: stack1[32 * j:32 * j + 32, ts(k, 128)],
                            h1_wc[j], 32 * j, n)

                # ---- propagation step 2 ----
                pt2 = prop_step(h1_wc, [0, 1, 2], strm2)

                # hop copy + mlp + LN stats, interleaved per chunk
                pm = [ps.tile([32, 512], F32, tag="ps", name=f"pm{_n}")
                      for _n in range(NCH)]
                for n in range(NCH):
                    nc.vector.tensor_copy(stack2[:, ts(n, 512)], pt2[n][0:96, :])
                    nc.tensor.matmul(pm[n][:], ga[:, ts(i, 32)],
                                     hid[0:32, ts(n, 512)], start=True, stop=False)
                    nc.tensor.matmul(pm[n][:], g1s[:, ts(i, 32)],
                                     stack1[:, ts(n, 512)], start=False, stop=False)
                    nc.tensor.matmul(pm[n][:], g2s[:, ts(i, 32)],
                                     stack2[:, ts(n, 512)], start=False, stop=True)
                    nc.vector.scalar_tensor_tensor(
                        out=pm[n][:], in0=pm[n][:],
                        scalar=gb[:, i:i + 1], in1=hid[0:32, ts(n, 512)],
                        op0=ALU.add, op1=ALU.add)
                    nc.vector.tensor_reduce(sums[:, n:n + 1], pm[n][:],
                                            mybir.AxisListType.X, ALU.add)
                    sq = scp.tile([32, 512], BF16, tag="tmp", name="sq")
                    nc.scalar.activation(sq[:], pm[n][:], AF.Square,
                                         accum_out=sums[:, NCH + n:NCH + n + 1])

                # ---- layernorm scalar chain ----
                tot = wk.tile([1, 2 * NCH], F32, name=f"tot{i}")
                nc.gpsimd.tensor_reduce(tot[:], sums[:], mybir.AxisListType.C, ALU.add)
                nc.vector.tensor_reduce(stat[:, 0:1], tot[:, 0:NCH],
                                        mybir.AxisListType.X, ALU.add)
                nc.vector.tensor_reduce(stat[:, 1:2], tot[:, NCH:2 * NCH],
                                        mybir.AxisListType.X, ALU.add)
                nc.vector.tensor_scalar_mul(stat[:, 0:1], stat[:, 0:1], 1.0 / (C * N))
                nc.vector.tensor_scalar_mul(stat[:, 1:2], stat[:, 1:2], 1.0 / (C * N))
                nc.vector.tensor_mul(stat[:, 2:3], stat[:, 0:1], stat[:, 0:1])
                nc.vector.tensor_sub(stat[:, 1:2], stat[:, 1:2], stat[:, 2:3])
                nc.scalar.activation(stat[:, 1:2], stat[:, 1:2], AF.Sqrt, bias=eps_t[:])
                nc.vector.reciprocal(stat[:, 1:2], stat[:, 1:2])
                pb = ps.tile([32, 2], F32, tag="ps", name="pb")
                nc.tensor.matmul(pb[:], ones1[:], stat[:, 0:2], start=True, stop=True)
                nc.vector.tensor_copy(bc[:], pb[:])

                # ---- normalize + (affine) + relu -> hid[0:32], + x transpose ----
                for n in range(NCH):
                    if has_affine:
                        tmp = scp.tile([32, 512], BF16, tag="tmp", name="tmp")
                        nc.vector.tensor_scalar(
                            out=tmp[:], in0=pm[n][:],
                            scalar1=bc[:, 0:1], scalar2=bc[:, 1:2],
                            op0=ALU.subtract, op1=ALU.mult)
                        nc.vector.tensor_mul(tmp[:], tmp[:],
                                             nw[:, bass.ds(i * N + n * 512, 512)])
                        nc.vector.tensor_add(tmp[:], tmp[:],
                                             nb[:, bass.ds(i * N + n * 512, 512)])
                        nc.vector.tensor_scalar_max(hid[0:32, ts(n, 512)], tmp[:], 0.0)
                    else:
                        tmp = scp.tile([32, 512], BF16, tag="tmp", name="tmp")
                        nc.vector.tensor_scalar(
                            out=tmp[:], in0=pm[n][:],
                            scalar1=bc[:, 0:1], scalar2=bc[:, 1:2],
                            op0=ALU.subtract, op1=ALU.mult)
                        nc.vector.tensor_scalar_max(hid[0:32, ts(n, 512)], tmp[:], 0.0)
                    if i < LAYERS - 1:
                        transpose_grp(lambda k: hid[0:32, ts(k, 128)], x_wc, 0, n)

            # ---- final head: out = out_acc + EWE @ x3 + cb ----
            for n in range(NCH):
                pk = ps.tile([T, 512], F32, tag="ps", name="pk")
                nc.tensor.matmul(pk[:], ewe[:], hid[0:32, ts(n, 512)],
                                 start=True, stop=True)
                nc.vector.scalar_tensor_tensor(
                    out=out_acc[:, ts(n, 512)], in0=pk[:], scalar=cb[:],
                    in1=out_acc[:, ts(n, 512)], op0=ALU.add, op1=ALU.add)
            nc.sync.dma_start(out_d[:], out_acc[:])

    return nc


def _prep(inputs):
    """Host-side preprocessing -> per-core input maps."""
    f32 = np.float32
    x = inputs['x'].astype(f32).reshape(B, C, N)
    dy = inputs['dy_graph'].astype(f32)
    S_ = inputs['static_graph'].astype(f32)
    sp = inputs['spatial_emb'].astype(f32).reshape(B, 32, N)
    td = inputs['temporal_d_emb'].astype(f32).reshape(B, 32, N)
    tw = inputs['temporal_w_emb'].astype(f32).reshape(B, 32, N)

    sc = np.float32(1.0 - ALPHA)
    # static: rhs0[w,v] = (S^T + I)[w,v] * (1-a)/r0[v],  r0 = S.sum(1)+1
    r0 = S_.sum(1) + 1.0
    adj0 = ((S_.T + np.eye(N, dtype=f32)) * (sc / r0)[None, :]).astype(NBF)
    adj1 = np.empty((B, N, N), NBF)
    adj2 = np.empty((B, N, N), NBF)
    for b in range(B):
        d = dy[b]
        r1 = d.sum(1) + 1.0
        r2 = d.sum(0) + 1.0
        dT = np.ascontiguousarray(d.T)
        adj1[b] = ((dT + np.eye(N, dtype=f32)) * (sc / r1)[None, :]).astype(NBF)
        adj2[b] = ((d + np.eye(N, dtype=f32)) * (sc / r2)[None, :]).astype(NBF)

    id3 = np.zeros((96, 32), f32)
    for j in range(3):
        id3[32 * j:32 * j + 32] = np.eye(32)
    alphaI = (ALPHA * np.eye(32, dtype=f32)).astype(NBF)
    id3 = id3.astype(NBF)

    # TCN weights: lhsT = W^T laid out [cin, (layer,unit,fg)*cout]
    tcnW = np.zeros((H, LAYERS * 4 * H), f32)
    tcnB = np.zeros((H, LAYERS * 4), f32)
    for i in range(LAYERS):
        for u in range(NUM_TCN):
            tcnW[:, (i * 4 + 2 * u) * H:(i * 4 + 2 * u + 1) * H] = \
                inputs['enc_Wf'][i, u].astype(f32).T
            tcnW[:, (i * 4 + 2 * u + 1) * H:(i * 4 + 2 * u + 2) * H] = \
                inputs['enc_Wg'][i, u].astype(f32).T
            tcnB[:, i * 4 + 2 * u] = inputs['enc_bf'][i, u].astype(f32)
            tcnB[:, i * 4 + 2 * u + 1] = inputs['enc_bg'][i, u].astype(f32)

    gW = [inputs['g0_W'].astype(f32), inputs['g1_W'].astype(f32),
          inputs['g2_W'].astype(f32)]
    gB = [inputs['g0_b'].astype(f32), inputs['g1_b'].astype(f32),
          inputs['g2_b'].astype(f32)]
    g1s = np.zeros((96, LAYERS * 32), f32)
    g2s = np.zeros((96, LAYERS * 32), f32)
    ga = np.zeros((32, LAYERS * 32), f32)
    gb = np.zeros((32, LAYERS), f32)
    for i in range(LAYERS):
        for k in range(3):   # branch k: 0 static, 1 dy, 2 dyT
            g1s[32 * k:32 * k + 32, 32 * i:32 * i + 32] = gW[k][i][:, 32:64].T
            g2s[32 * k:32 * k + 32, 32 * i:32 * i + 32] = gW[k][i][:, 64:96].T
            ga[:, 32 * i:32 * i + 32] += gW[k][i][:, 0:32].T
            gb[:, i] += gB[k][i]

    endW = inputs['end_W'].astype(f32)
    ew = np.zeros((H, LAYERS * T), f32)
    for i in range(LAYERS):
        ew[:, i * T:(i + 1) * T] = (endW @ inputs['skip_W'][i].astype(f32)).T
    ewe = (endW @ inputs['skipE_W'].astype(f32)).T
    cb = (endW @ (inputs['skip_b'].astype(f32).sum(0)
                  + inputs['skipE_b'].astype(f32))
          + inputs['end_b'].astype(f32)).reshape(T, 1)

    nw = inputs['norm_w'].astype(f32).reshape(LAYERS, C, N)
    nbb = inputs['norm_b'].astype(f32).reshape(LAYERS, C, N)
    has_affine = not (np.all(nw == 1.0) and np.all(nbb == 0.0))

    shared = {
        "adj0": adj0, "id3": id3, "alphaI": alphaI,
        "tcnW": tcnW.astype(NBF), "tcnB": tcnB,
        "g1s": g1s.astype(NBF), "g2s": g2s.astype(NBF),
        "ga": ga.astype(NBF), "gb": gb,
        "ew": ew.astype(NBF), "ewe": ewe.astype(NBF), "cb": cb,
        "embs": None,  # per-core below
    }
    if has_affine:
        shared["nw"] = np.concatenate([nw[i] for i in range(LAYERS)], 1).astype(NBF)
        shared["nb"] = np.concatenate([nbb[i] for i in range(LAYERS)], 1).astype(NBF)

    in_maps = []
    for b in range(B):
        m = dict(shared)
        m["embs"] = np.concatenate([sp[b], td[b], tw[b]], 0).astype(NBF)
        m["x0"] = x[b].astype(NBF)
        m["x0T"] = np.ascontiguousarray(x[b].T).astype(NBF)
        m["adj1"] = adj1[b]
        m["adj2"] = adj2[b]
        in_maps.append(m)
    return in_maps, has_affine


LAST_EXEC_NS = None


def _install_profile_hook():
    import types
    import antenv
    if 'antenv.axon_hooks' not in sys.modules:
        mod = types.ModuleType('antenv.axon_hooks')
        holder = {}
        mod.set_axon_ntff_profile_hook = lambda h: holder.__setitem__('h', h)
        mod.get_axon_ntff_profile_hook = lambda: holder.get('h')
        sys.modules['antenv.axon_hooks'] = mod
        antenv.axon_hooks = mod
        from trn_agent_boot.trn_boot import _ntff_profile_via_ctypes
        mod.set_axon_ntff_profile_hook(
            _ntff_profile_via_ctypes('/opt/axon/libaxon_pjrt.so'))
    import concourse.bass_utils as bu
    bu.upload_artifacts = lambda tmpdir: tmpdir


def kernel(**inputs):
    global LAST_EXEC_NS
    in_maps, has_affine = _prep(inputs)
    if has_affine not in _prog_cache:
        _prog_cache[has_affine] = _build(has_affine)
    nc = _prog_cache[has_affine]
    trace = bool(os.environ.get("KERNEL_TRACE"))
    if trace:
        _install_profile_hook()
    res = run_bass_kernel_spmd(nc, in_maps, core_ids=list(range(B)), trace=trace)
    LAST_EXEC_NS = res.exec_time_ns
    out = np.stack([res.results[b]["out"] for b in range(B)], 0)
    return out.reshape(B, T, N, 1).astype(np.float32)
